# revision 1
# baseline (speedup 1.0000x reference)
"""DDNLoss (depth distribution network focal loss) on 8 trn2 NeuronCores.

Data-parallel over B (1 image per core, B=8). Each core:
  1. Rasterizes its 32 boxes into a min-depth map (96,312):
     per box, PE broadcasts the box's column-value row (depth where the
     box covers the column, +inf elsewhere) to 96 partitions via a K=1
     fp32 matmul (exact), then one fused DVE op does
     dmin = min(dmin, max(colval_bcast, rowpen_scalar)).
  2. Computes LID bin targets t(h,w) and fg weights on-chip, bounces
     them through DRAM to a (128, 234) pixel-partition "slot" layout
     (slot (i,g) <-> pixel 128*g + i).
  3. Streams depth_logits (81, 29952) in 13 contiguous chunks. Each
     128-pixel group is PE-transposed to (128, 81); ACT computes exp
     with accum_out giving sum_c exp directly; one fused DVE op
     (iota==t)*exp with accum_out gives exp(logit[target]).
  4. Focal-loss epilogue on (128,234) + free-dim accumulation ->
     per-partition partial sums (128,1).
Host sums the 8x128 partials (the "all-reduce") -> scalar loss.
"""

import numpy as np
from contextlib import ExitStack

import concourse.bass as bass
import concourse.bacc as bacc_mod
import concourse.tile as tile
import concourse.mybir as mybir
from concourse import masks
from concourse.bass_utils import run_bass_kernel_spmd

# Problem constants (hardcoded per contract)
B, C, H, W, N = 8, 81, 96, 312, 32
HW = H * W                      # 29952
CHUNK = 2304                    # pixels per streamed chunk
NCHUNK = HW // CHUNK            # 13
GP = CHUNK // 128               # 18 pixel-groups of 128 per chunk
NGRP = HW // 128                # 234

ALPHA = 0.25
D_MIN, D_MAX, NUM_BINS = 0.001, 60.0, 80
BIN_SIZE = 2.0 * (D_MAX - D_MIN) / (NUM_BINS * (1 + NUM_BINS))
K1 = 8.0 / BIN_SIZE             # sqrt arg scale
B1 = 1.0 - K1 * D_MIN           # sqrt arg bias
BIG = 1.0e30
C0 = -ALPHA / float(B * HW)     # fold -alpha and global pixel normalizer
# float->int32 conversion rounding on DVE: True = round-to-nearest (cast
# idx-0.5), False = truncate (cast idx directly). Flip if probe mismatches.
CAST_RNE = True

LAST_RESULTS = None


def build_program(ablate=()):
    f32 = mybir.dt.float32
    i32 = mybir.dt.int32
    Alu = mybir.AluOpType
    Act = mybir.ActivationFunctionType

    nc = bacc_mod.Bacc("TRN2", target_bir_lowering=False)
    logits = nc.dram_tensor("logits", [C, HW], f32, kind="ExternalInput")
    rowpen = nc.dram_tensor("rowpen", [H, N], f32, kind="ExternalInput")
    colval = nc.dram_tensor("colval", [N, W], f32, kind="ExternalInput")
    iotaf = nc.dram_tensor("iotaf", [128, C], f32, kind="ExternalInput")
    ones96 = nc.dram_tensor("ones96", [1, H], f32, kind="ExternalInput")
    partial = nc.dram_tensor("partial", [128, 1], f32, kind="ExternalOutput")
    tprobe = nc.dram_tensor("tprobe", [1, HW], f32, kind="ExternalOutput")

    with ExitStack() as ctx:
        tc = ctx.enter_context(tile.TileContext(nc))
        consts = ctx.enter_context(tc.tile_pool(name="consts", bufs=1))
        rast = ctx.enter_context(tc.tile_pool(name="rast", bufs=1))
        ts_pool = ctx.enter_context(tc.tile_pool(name="tstage", bufs=1))
        lg = ctx.enter_context(tc.tile_pool(name="lg", bufs=2))
        ex = ctx.enter_context(tc.tile_pool(name="ex", bufs=4))
        pr = ctx.enter_context(tc.tile_pool(name="pr", bufs=4))
        fin = ctx.enter_context(tc.tile_pool(name="fin", bufs=1))
        psb = ctx.enter_context(tc.tile_pool(name="psb", bufs=3, space="PSUM"))
        pst_pool = ctx.enter_context(tc.tile_pool(name="pst", bufs=4, space="PSUM"))
        dr = ctx.enter_context(tc.tile_pool(name="dr", bufs=1, space="DRAM"))

        # ---- constants
        zero128 = consts.tile([128, 1], f32)
        nc.vector.memset(zero128[:], 0.0)
        nc.const_aps.aps[(f32, 0.0)] = zero128[:]
        b1t = consts.tile([128, 1], f32)
        nc.vector.memset(b1t[:], B1)
        nc.const_aps.aps[(f32, B1)] = b1t[:]

        c_iota = consts.tile([128, C], f32)
        nc.sync.dma_start(c_iota[:], iotaf[:, :])
        c_ones96 = consts.tile([1, H], f32)
        nc.sync.dma_start(c_ones96[:], ones96[:, :])
        c_rowpen = consts.tile([H, N], f32)
        nc.sync.dma_start(c_rowpen[:], rowpen[:, :])
        c_cv = []
        for n in range(N):
            cvn = consts.tile([1, W], f32, tag=f"cv{n}")
            nc.sync.dma_start(cvn[:], colval[n:n + 1, :])
            c_cv.append(cvn)
        ident = consts.tile([128, 128], f32)
        masks.make_identity(nc, ident[:])

        # ---- rasterize: dmin(h,w) = min_n max(rowpen(h,n), colval(n,w))
        dmin = rast.tile([H, W], f32)
        nc.vector.memset(dmin[:], BIG)
        for n in range(N):
            bc = psb.tile([H, W], f32, tag="bc")
            nc.tensor.matmul(bc[:], c_ones96[:, :],
                             c_cv[n][0:1, :],
                             start=True, stop=True)
            # dmin = min(dmin, max(bc, rowpen[:, n]))
            nc.vector.scalar_tensor_tensor(
                out=dmin[:], in0=bc[:], scalar=c_rowpen[:, n:n + 1], in1=dmin[:],
                op0=Alu.max, op1=Alu.min)

        # ---- per-pixel targets in raster layout (96,312)
        fg = ts_pool.tile([H, W], f32)
        nc.vector.tensor_scalar(out=fg[:], in0=dmin[:], scalar1=BIG * 0.5,
                                scalar2=None, op0=Alu.is_lt)
        wgt = ts_pool.tile([H, W], f32)
        nc.vector.tensor_scalar(out=wgt[:], in0=fg[:], scalar1=12.0,
                                scalar2=1.0, op0=Alu.mult, op1=Alu.add)
        deff = ts_pool.tile([H, W], f32)
        nc.vector.tensor_tensor(out=deff[:], in0=dmin[:], in1=fg[:], op=Alu.mult)
        # idx = 0.5*sqrt(K1*d + B1) - 0.5
        sq = ts_pool.tile([H, W], f32)
        nc.scalar.activation(sq[:], deff[:], Act.Sqrt, bias=B1, scale=K1)
        idx = ts_pool.tile([H, W], f32)
        nc.vector.tensor_scalar(out=idx[:], in0=sq[:], scalar1=0.5,
                                scalar2=-0.5, op0=Alu.mult, op1=Alu.add)
        neg = ts_pool.tile([H, W], f32)
        nc.vector.tensor_scalar(out=neg[:], in0=idx[:], scalar1=0.0,
                                scalar2=None, op0=Alu.is_lt)
        idxc = ts_pool.tile([H, W], f32)
        if CAST_RNE:
            nc.vector.tensor_scalar(out=idxc[:], in0=idx[:], scalar1=80.0,
                                    scalar2=-0.5, op0=Alu.min, op1=Alu.add)
        else:
            nc.vector.tensor_scalar(out=idxc[:], in0=idx[:], scalar1=80.0,
                                    scalar2=None, op0=Alu.min)
        ti = ts_pool.tile([H, W], i32)
        nc.vector.tensor_copy(out=ti[:], in_=idxc[:])
        tf = ts_pool.tile([H, W], f32)
        nc.vector.tensor_copy(out=tf[:], in_=ti[:])
        # t = tf + neg*(80 - tf)   (idx<0 -> bin 80)
        d80 = ts_pool.tile([H, W], f32)
        nc.vector.tensor_scalar(out=d80[:], in0=tf[:], scalar1=-1.0,
                                scalar2=80.0, op0=Alu.mult, op1=Alu.add)
        nd = ts_pool.tile([H, W], f32)
        nc.vector.tensor_tensor(out=nd[:], in0=neg[:], in1=d80[:], op=Alu.mult)
        tt_ = ts_pool.tile([H, W], f32)
        nc.vector.tensor_tensor(out=tt_[:], in0=tf[:], in1=nd[:], op=Alu.add)

        nc.sync.dma_start(tprobe[0:1, :], tt_[:])

        # ---- bounce t and w to DRAM (flat pixel order), reload in slot
        # layout: slot (i, g) <- pixel 128*g + i
        tscr = dr.tile([NGRP, 128], f32)
        nc.sync.dma_start(tscr[:, :], tt_[:])
        wscr = dr.tile([NGRP, 128], f32)
        nc.sync.dma_start(wscr[:, :], wgt[:])
        t_slot = fin.tile([128, NGRP], f32)
        nc.sync.dma_start(t_slot[:], tscr[:, :].rearrange("c i -> i c"))
        w_slot = fin.tile([128, NGRP], f32)
        nc.sync.dma_start(w_slot[:], wscr[:, :].rearrange("c i -> i c"))

        # ---- stream logits; per 128-pixel group: PE transpose ->
        # (128, 81), exp+rowsum on ACT, (iota==t)*exp rowsum on DVE
        s128 = fin.tile([128, NGRP], f32)   # sum_c exp
        e128 = fin.tile([128, NGRP], f32)   # exp(logit[target])
        for j in range(NCHUNK):
            sl = slice(j * CHUNK, (j + 1) * CHUNK)
            L = lg.tile([C, CHUNK], f32, tag="L")
            nc.sync.dma_start(L[:], logits[:, sl])
            for k in range(GP):
                g = j * GP + k
                ksl = slice(k * 128, (k + 1) * 128)
                if "tp" in ablate:
                    continue
                pst = pst_pool.tile([128, C], f32, tag="pst")
                nc.tensor.transpose(pst[:], L[:, ksl], ident[0:C, 0:C])
                if "exp" in ablate:
                    continue
                expt = ex.tile([128, C], f32, tag="expt")
                nc.scalar.activation(expt[:], pst[:], Act.Exp,
                                     accum_out=s128[:, g:g + 1])
                if "prod" in ablate:
                    continue
                prod = pr.tile([128, C], f32, tag="prod")
                nc.vector.scalar_tensor_tensor(
                    out=prod[:], in0=c_iota[:], scalar=t_slot[:, g:g + 1],
                    in1=expt[:], op0=Alu.is_equal, op1=Alu.mult,
                    accum_out=e128[:, g:g + 1])

        # ---- focal epilogue on (128, 234)
        rS = fin.tile([128, NGRP], f32)
        nc.vector.reciprocal(rS[:], s128[:])
        p = fin.tile([128, NGRP], f32)
        nc.vector.tensor_tensor(out=p[:], in0=e128[:], in1=rS[:], op=Alu.mult)
        logp = fin.tile([128, NGRP], f32)
        nc.scalar.activation(logp[:], p[:], Act.Ln)
        om = fin.tile([128, NGRP], f32)
        nc.vector.tensor_scalar(out=om[:], in0=p[:], scalar1=-1.0,
                                scalar2=1.0, op0=Alu.mult, op1=Alu.add)
        om2 = fin.tile([128, NGRP], f32)
        nc.vector.tensor_tensor(out=om2[:], in0=om[:], in1=om[:], op=Alu.mult)
        t2 = fin.tile([128, NGRP], f32)
        nc.vector.scalar_tensor_tensor(
            out=t2[:], in0=om2[:], scalar=C0, in1=logp[:],
            op0=Alu.mult, op1=Alu.mult)
        fs = fin.tile([128, NGRP], f32)
        acc = fin.tile([128, 1], f32)
        nc.vector.scalar_tensor_tensor(
            out=fs[:], in0=t2[:], scalar=0.0, in1=w_slot[:],
            op0=Alu.add, op1=Alu.mult, accum_out=acc[:])
        nc.sync.dma_start(partial[:, :], acc[:])

    nc.compile()
    return nc


_CACHE = {}


def _get_program():
    if "nc" not in _CACHE:
        _CACHE["nc"] = build_program()
    return _CACHE["nc"]


def kernel(depth_logits, gt_boxes2d, num_gt_per_img, gt_center_depth):
    global LAST_RESULTS
    dl = np.ascontiguousarray(np.asarray(depth_logits, dtype=np.float32))
    assert dl.shape == (B, C, H, W), dl.shape
    n_gt = int(num_gt_per_img)
    assert n_gt == N, n_gt
    boxes = np.asarray(gt_boxes2d, dtype=np.float32)
    depth = np.asarray(gt_center_depth, dtype=np.float32)

    u1 = np.floor(boxes[:, 0]).astype(np.int32)
    v1 = np.floor(boxes[:, 1]).astype(np.int32)
    u2 = np.ceil(boxes[:, 2]).astype(np.int32)
    v2 = np.ceil(boxes[:, 3]).astype(np.int32)
    rows = np.arange(H)[:, None]
    cols = np.arange(W)[None, :]
    iota = np.ascontiguousarray(
        np.tile(np.arange(C, dtype=np.float32), (128, 1)))
    ones = np.ones((1, H), dtype=np.float32)

    logits_flat = dl.reshape(B, C, HW)
    in_maps = []
    for b in range(B):
        sl = slice(b * N, (b + 1) * N)
        bv1, bv2, bu1, bu2, d = v1[sl], v2[sl], u1[sl], u2[sl], depth[sl]
        rp = np.where((rows >= bv1[None, :]) & (rows < bv2[None, :]),
                      0.0, BIG).astype(np.float32)              # (H, N)
        cv = np.where((cols >= bu1[:, None]) & (cols < bu2[:, None]),
                      d[:, None], BIG).astype(np.float32)       # (N, W)
        in_maps.append({
            "logits": np.ascontiguousarray(logits_flat[b]),
            "rowpen": np.ascontiguousarray(rp),
            "colval": np.ascontiguousarray(cv),
            "iotaf": iota,
            "ones96": ones,
        })

    nc = _get_program()
    res = run_bass_kernel_spmd(nc, in_maps, core_ids=list(range(B)))
    LAST_RESULTS = res
    total = np.float64(0.0)
    for r in res.results:
        total += np.asarray(r["partial"], dtype=np.float64).sum()
    return np.float32(total)


if __name__ == "__main__":
    import tempfile
    from concourse.bass_utils import compile_bass_kernel
    compile_bass_kernel(_get_program(), tempfile.mkdtemp())
    print("COMPILE OK")



# revision 5
# speedup vs baseline: 1.5792x; 1.5792x over previous
"""DDNLoss (depth distribution network focal loss) on 8 trn2 NeuronCores.

Data-parallel over B (1 image per core, B=8). Redesigned C-major pipeline
(v2) to kill the per-group instruction overhead of v1:

  1. Rasterize min-depth map (96,312) via 32 K=1 PE matmuls + DVE merges
     (unchanged from v1), then LID bin targets t and gather indices
     idx = t*HW + p on-chip.
  2. idx bounces through DRAM into column-slot layout (128,234)
     (slot (i,g) <-> pixel 128g+i), then ONE gpsimd indirect DMA gathers
     l_t[p] = logits[t_p, p] directly into (128,234) slot layout.
  3. logits stream C-major in 13 chunks (81,2304); ACT computes
     e = exp(l) -> bf16 in bulk (13 big instructions).
  4. PE transposes e into PSUM banks (12 groups of (128,81) bf16 per
     bank); one DVE 3D tensor_reduce per bank yields S = sum_c e in
     dense slot layout (128,234) -- 20 instructions instead of 234.
  5. Dense epilogue: lnp = l_t - ln S; p = exp(lnp);
     loss = C0*w*(1-p)^2*lnp with w reconstructed from idx; free-dim
     accumulate -> per-partition partials (128,1).
Host sums the 8x128 partials -> scalar loss.
"""

import numpy as np
from contextlib import ExitStack

import concourse.bass as bass
import concourse.bacc as bacc_mod
import concourse.tile as tile
import concourse.mybir as mybir
from concourse import masks
from concourse.bass_utils import run_bass_kernel_spmd

# Problem constants (hardcoded per contract)
B, C, H, W, N = 8, 81, 96, 312, 32
HW = H * W                      # 29952
CHUNK = 2304                    # pixels per streamed chunk
NCHUNK = HW // CHUNK            # 13
GP = CHUNK // 128               # 18 pixel-groups of 128 per chunk
NGRP = HW // 128                # 234
GPB = 6                         # groups per PSUM bank (6*81*4B = 1944B)
NBANK = (NGRP + GPB - 1) // GPB  # 20 (last bank 6 groups)

ALPHA = 0.25
D_MIN, D_MAX, NUM_BINS = 0.001, 60.0, 80
BIN_SIZE = 2.0 * (D_MAX - D_MIN) / (NUM_BINS * (1 + NUM_BINS))
K1 = 8.0 / BIN_SIZE             # sqrt arg scale
B1 = 1.0 - K1 * D_MIN           # sqrt arg bias
BIG = 1.0e30
C0 = -ALPHA / float(B * HW)     # fold -alpha and global pixel normalizer
CAST_RNE = True

LAST_RESULTS = None


def build_program():
    f32 = mybir.dt.float32
    bf16 = mybir.dt.bfloat16
    i32 = mybir.dt.int32
    Alu = mybir.AluOpType
    Act = mybir.ActivationFunctionType
    Ax = mybir.AxisListType

    nc = bacc_mod.Bacc("TRN2", target_bir_lowering=False)
    # logits kept flat so the gather can index single fp32 elements
    logits = nc.dram_tensor("logits", [C * HW, 1], f32, kind="ExternalInput")
    rowpen = nc.dram_tensor("rowpen", [H, N], f32, kind="ExternalInput")
    colval = nc.dram_tensor("colval", [N, W], f32, kind="ExternalInput")
    ones96 = nc.dram_tensor("ones96", [1, H], f32, kind="ExternalInput")
    pixiota = nc.dram_tensor("pixiota", [H, W], f32, kind="ExternalInput")
    pcolslot = nc.dram_tensor("pcolslot", [128, NGRP], f32, kind="ExternalInput")
    partial = nc.dram_tensor("partial", [128, 1], f32, kind="ExternalOutput")
    tprobe = nc.dram_tensor("tprobe", [1, HW], f32, kind="ExternalOutput")

    lg_full = logits[:, :].rearrange("(c p) o -> c (p o)", c=C)  # (81, HW)

    with ExitStack() as ctx:
        tc = ctx.enter_context(tile.TileContext(nc))
        consts = ctx.enter_context(tc.tile_pool(name="consts", bufs=1))
        rast = ctx.enter_context(tc.tile_pool(name="rast", bufs=1))
        ts_pool = ctx.enter_context(tc.tile_pool(name="tstage", bufs=1))
        lg = ctx.enter_context(tc.tile_pool(name="lg", bufs=3))
        ep = ctx.enter_context(tc.tile_pool(name="ep", bufs=3))
        fin = ctx.enter_context(tc.tile_pool(name="fin", bufs=1))
        psb = ctx.enter_context(tc.tile_pool(name="psb", bufs=2, space="PSUM"))
        pst_pool = ctx.enter_context(tc.tile_pool(name="pst", bufs=4, space="PSUM"))
        dr = ctx.enter_context(tc.tile_pool(name="dr", bufs=1, space="DRAM"))

        # ---- constants
        zero128 = consts.tile([128, 1], f32)
        nc.vector.memset(zero128[:], 0.0)
        nc.const_aps.aps[(f32, 0.0)] = zero128[:]
        b1t = consts.tile([128, 1], f32)
        nc.vector.memset(b1t[:], B1)
        nc.const_aps.aps[(f32, B1)] = b1t[:]

        c_ones96 = consts.tile([1, H], f32)
        nc.sync.dma_start(c_ones96[:], ones96[:, :])
        c_rowpen = consts.tile([H, N], f32)
        nc.sync.dma_start(c_rowpen[:], rowpen[:, :])
        c_pix = consts.tile([H, W], f32)
        nc.sync.dma_start(c_pix[:], pixiota[:, :])
        c_pcs = consts.tile([128, NGRP], f32)
        nc.sync.dma_start(c_pcs[:], pcolslot[:, :])
        c_cv = []
        for n in range(N):
            cvn = consts.tile([1, W], f32, tag=f"cv{n}")
            nc.sync.dma_start(cvn[:], colval[n:n + 1, :])
            c_cv.append(cvn)
        ident = consts.tile([128, 128], f32)
        masks.make_identity(nc, ident[:])
        identb = consts.tile([128, 128], bf16)
        nc.vector.tensor_copy(out=identb[:], in_=ident[:])

        # ---- rasterize: dmin(h,w) = min_n max(rowpen(h,n), colval(n,w))
        dmin = rast.tile([H, W], f32)
        nc.vector.memset(dmin[:], BIG)
        for n in range(N):
            bc = psb.tile([H, W], f32, tag="bc")
            nc.tensor.matmul(bc[:], c_ones96[:, :], c_cv[n][0:1, :],
                             start=True, stop=True)
            nc.vector.scalar_tensor_tensor(
                out=dmin[:], in0=bc[:], scalar=c_rowpen[:, n:n + 1], in1=dmin[:],
                op0=Alu.max, op1=Alu.min)

        # ---- per-pixel targets in raster layout (96,312)
        fg = ts_pool.tile([H, W], f32)
        nc.vector.tensor_scalar(out=fg[:], in0=dmin[:], scalar1=BIG * 0.5,
                                scalar2=None, op0=Alu.is_lt)
        deff = ts_pool.tile([H, W], f32)
        nc.vector.tensor_tensor(out=deff[:], in0=dmin[:], in1=fg[:], op=Alu.mult)
        sq = ts_pool.tile([H, W], f32)
        nc.scalar.activation(sq[:], deff[:], Act.Sqrt, bias=B1, scale=K1)
        idx = ts_pool.tile([H, W], f32)
        nc.vector.tensor_scalar(out=idx[:], in0=sq[:], scalar1=0.5,
                                scalar2=-0.5, op0=Alu.mult, op1=Alu.add)
        neg = ts_pool.tile([H, W], f32)
        nc.vector.tensor_scalar(out=neg[:], in0=idx[:], scalar1=0.0,
                                scalar2=None, op0=Alu.is_lt)
        idxc = ts_pool.tile([H, W], f32)
        if CAST_RNE:
            nc.vector.tensor_scalar(out=idxc[:], in0=idx[:], scalar1=80.0,
                                    scalar2=-0.5, op0=Alu.min, op1=Alu.add)
        else:
            nc.vector.tensor_scalar(out=idxc[:], in0=idx[:], scalar1=80.0,
                                    scalar2=None, op0=Alu.min)
        ti = ts_pool.tile([H, W], i32)
        nc.vector.tensor_copy(out=ti[:], in_=idxc[:])
        tf = ts_pool.tile([H, W], f32)
        nc.vector.tensor_copy(out=tf[:], in_=ti[:])
        # t = tf + neg*(80 - tf)   (idx<0 -> bin 80)
        d80 = ts_pool.tile([H, W], f32)
        nc.vector.tensor_scalar(out=d80[:], in0=tf[:], scalar1=-1.0,
                                scalar2=80.0, op0=Alu.mult, op1=Alu.add)
        nd = ts_pool.tile([H, W], f32)
        nc.vector.tensor_tensor(out=nd[:], in0=neg[:], in1=d80[:], op=Alu.mult)
        tt_ = ts_pool.tile([H, W], f32)
        nc.vector.tensor_tensor(out=tt_[:], in0=tf[:], in1=nd[:], op=Alu.add)

        nc.sync.dma_start(tprobe[0:1, :], tt_[:])

        # gather index per pixel: gidx = t*HW + p   (exact in fp32)
        gidxf = ts_pool.tile([H, W], f32)
        nc.vector.scalar_tensor_tensor(
            out=gidxf[:], in0=tt_[:], scalar=float(HW), in1=c_pix[:],
            op0=Alu.mult, op1=Alu.add)
        gidx = ts_pool.tile([H, W], i32)
        nc.vector.tensor_copy(out=gidx[:], in_=gidxf[:])

        # ---- bounce gidx to DRAM (flat pixel order), reload in col-slot
        # layout: slot (i, g) <- pixel 128*g + i
        iscr = dr.tile([NGRP, 128], i32)
        nc.sync.dma_start(iscr[:, :], gidx[:])
        idx_slot = fin.tile([128, NGRP], i32)
        nc.sync.dma_start(idx_slot[:], iscr[:, :].rearrange("c i -> i c"))

        # ---- ONE indirect DMA: l_t[i,g] = logits_flat[idx_slot[i,g]]
        lt_slot = fin.tile([128, NGRP], f32)
        nc.gpsimd.indirect_dma_start(
            out=lt_slot[:],
            out_offset=None,
            in_=logits[:, :],
            in_offset=bass.IndirectOffsetOnAxis(ap=idx_slot[:], axis=0),
        )

        # ---- stream logits C-major; exp -> bf16; PE-transpose into PSUM
        # banks; per-bank 3D reduce -> S (128, 234)
        S_dense = fin.tile([128, NGRP], f32)
        bank = None
        for j in range(NCHUNK):
            sl = slice(j * CHUNK, (j + 1) * CHUNK)
            L = lg.tile([C, CHUNK], f32, tag="L")
            nc.sync.dma_start(L[:], lg_full[:, sl])
            e = ep.tile([C, CHUNK], f32, tag="e")
            nc.scalar.activation(e[:], L[:], Act.Exp)

            # transpose all groups of chunk j ([18j, 18j+18)) into PSUM
            # banks of 12 groups; reduce each bank when it fills
            for m in range(GP):
                g = j * GP + m
                k = g % GPB
                if k == 0:
                    bank = pst_pool.tile([128, GPB * C], f32, tag="bank")
                nc.tensor.transpose(bank[:, k * C:(k + 1) * C],
                                    e[:, m * 128:(m + 1) * 128],
                                    ident[0:C, 0:C])
                if k == GPB - 1 or g == NGRP - 1:
                    ng = k + 1
                    g0 = g - k
                    nc.vector.tensor_reduce(
                        out=S_dense[:, g0:g0 + ng],
                        in_=bank[:, 0:ng * C].rearrange(
                            "p (g c) -> p g c", g=ng),
                        axis=Ax.X, op=Alu.add)

        # ---- dense epilogue on (128, 234)
        lnS = fin.tile([128, NGRP], f32)
        nc.scalar.activation(lnS[:], S_dense[:], Act.Ln)
        lnp = fin.tile([128, NGRP], f32)
        nc.vector.tensor_tensor(out=lnp[:], in0=lt_slot[:], in1=lnS[:],
                                op=Alu.subtract)
        p = fin.tile([128, NGRP], f32)
        nc.scalar.activation(p[:], lnp[:], Act.Exp)
        om = fin.tile([128, NGRP], f32)
        nc.vector.tensor_scalar(out=om[:], in0=p[:], scalar1=-1.0,
                                scalar2=1.0, op0=Alu.mult, op1=Alu.add)
        om2 = fin.tile([128, NGRP], f32)
        nc.vector.tensor_tensor(out=om2[:], in0=om[:], in1=om[:], op=Alu.mult)
        t3 = fin.tile([128, NGRP], f32)
        nc.vector.tensor_tensor(out=t3[:], in0=om2[:], in1=lnp[:], op=Alu.mult)

        # w from idx: t = (idx - p_colslot)/HW ; w = 1 + 12*(t < 79.5)
        idxf = fin.tile([128, NGRP], f32)
        nc.vector.tensor_copy(out=idxf[:], in_=idx_slot[:])
        tcol = fin.tile([128, NGRP], f32)
        nc.vector.scalar_tensor_tensor(
            out=tcol[:], in0=c_pcs[:], scalar=-1.0, in1=idxf[:],
            op0=Alu.mult, op1=Alu.add)
        wcol = fin.tile([128, NGRP], f32)
        nc.vector.tensor_scalar(out=wcol[:], in0=tcol[:],
                                scalar1=79.5 * float(HW), scalar2=None,
                                op0=Alu.is_lt)
        nc.vector.tensor_scalar(out=wcol[:], in0=wcol[:], scalar1=12.0,
                                scalar2=1.0, op0=Alu.mult, op1=Alu.add)

        acc = fin.tile([128, 1], f32)
        fs = fin.tile([128, NGRP], f32)
        nc.vector.scalar_tensor_tensor(
            out=fs[:], in0=t3[:], scalar=C0, in1=wcol[:],
            op0=Alu.mult, op1=Alu.mult, accum_out=acc[:])
        nc.sync.dma_start(partial[:, :], acc[:])

    nc.compile()
    return nc


_CACHE = {}


def _get_program():
    if "nc" not in _CACHE:
        _CACHE["nc"] = build_program()
    return _CACHE["nc"]


def kernel(depth_logits, gt_boxes2d, num_gt_per_img, gt_center_depth):
    global LAST_RESULTS
    dl = np.ascontiguousarray(np.asarray(depth_logits, dtype=np.float32))
    assert dl.shape == (B, C, H, W), dl.shape
    n_gt = int(num_gt_per_img)
    assert n_gt == N, n_gt
    boxes = np.asarray(gt_boxes2d, dtype=np.float32)
    depth = np.asarray(gt_center_depth, dtype=np.float32)

    u1 = np.floor(boxes[:, 0]).astype(np.int32)
    v1 = np.floor(boxes[:, 1]).astype(np.int32)
    u2 = np.ceil(boxes[:, 2]).astype(np.int32)
    v2 = np.ceil(boxes[:, 3]).astype(np.int32)
    rows = np.arange(H)[:, None]
    cols = np.arange(W)[None, :]
    ones = np.ones((1, H), dtype=np.float32)
    pix = np.arange(HW, dtype=np.float32).reshape(H, W)
    # pixel id of slot (i, g) = 128*g + i
    pcs = (np.arange(NGRP, dtype=np.float32)[None, :] * 128.0
           + np.arange(128, dtype=np.float32)[:, None])

    logits_flat = dl.reshape(B, C * HW, 1)
    in_maps = []
    for b in range(B):
        sl = slice(b * N, (b + 1) * N)
        bv1, bv2, bu1, bu2, d = v1[sl], v2[sl], u1[sl], u2[sl], depth[sl]
        rp = np.where((rows >= bv1[None, :]) & (rows < bv2[None, :]),
                      0.0, BIG).astype(np.float32)              # (H, N)
        cv = np.where((cols >= bu1[:, None]) & (cols < bu2[:, None]),
                      d[:, None], BIG).astype(np.float32)       # (N, W)
        in_maps.append({
            "logits": np.ascontiguousarray(logits_flat[b]),
            "rowpen": np.ascontiguousarray(rp),
            "colval": np.ascontiguousarray(cv),
            "ones96": ones,
            "pixiota": pix,
            "pcolslot": np.ascontiguousarray(pcs),
        })

    nc = _get_program()
    res = run_bass_kernel_spmd(nc, in_maps, core_ids=list(range(B)))
    LAST_RESULTS = res
    total = np.float64(0.0)
    for r in res.results:
        total += np.asarray(r["partial"], dtype=np.float64).sum()
    return np.float32(total)


if __name__ == "__main__":
    import tempfile
    from concourse.bass_utils import compile_bass_kernel
    compile_bass_kernel(_get_program(), tempfile.mkdtemp())
    print("COMPILE OK")


# revision 8
# speedup vs baseline: 1.7398x; 1.1017x over previous
"""DDNLoss (depth distribution network focal loss) on 8 trn2 NeuronCores.

Data-parallel over B (1 image per core, B=8). Redesigned C-major pipeline
(v2) to kill the per-group instruction overhead of v1:

  1. Rasterize min-depth map (96,312) via 32 K=1 PE matmuls + DVE merges
     (unchanged from v1), then LID bin targets t and gather indices
     idx = t*HW + p on-chip.
  2. idx bounces through DRAM into column-slot layout (128,234)
     (slot (i,g) <-> pixel 128g+i), then ONE gpsimd indirect DMA gathers
     l_t[p] = logits[t_p, p] directly into (128,234) slot layout.
  3. logits stream C-major in 13 chunks (81,2304); ACT computes
     e = exp(l) -> bf16 in bulk (13 big instructions).
  4. PE transposes e into PSUM banks (12 groups of (128,81) bf16 per
     bank); one DVE 3D tensor_reduce per bank yields S = sum_c e in
     dense slot layout (128,234) -- 20 instructions instead of 234.
  5. Dense epilogue: lnp = l_t - ln S; p = exp(lnp);
     loss = C0*w*(1-p)^2*lnp with w reconstructed from idx; free-dim
     accumulate -> per-partition partials (128,1).
Host sums the 8x128 partials -> scalar loss.
"""

import numpy as np
from contextlib import ExitStack

import concourse.bass as bass
import concourse.bacc as bacc_mod
import concourse.tile as tile
import concourse.mybir as mybir
from concourse import masks
from concourse.bass_utils import run_bass_kernel_spmd

# Problem constants (hardcoded per contract)
B, C, H, W, N = 8, 81, 96, 312, 32
HW = H * W                      # 29952
CHUNK = 2304                    # pixels per streamed chunk
NCHUNK = HW // CHUNK            # 13
GP = CHUNK // 128               # 18 pixel-groups of 128 per chunk
NGRP = HW // 128                # 234
GPB = 6                         # groups per PSUM bank (6*81*4B = 1944B)
NBANK = (NGRP + GPB - 1) // GPB  # 20 (last bank 6 groups)

ALPHA = 0.25
D_MIN, D_MAX, NUM_BINS = 0.001, 60.0, 80
BIN_SIZE = 2.0 * (D_MAX - D_MIN) / (NUM_BINS * (1 + NUM_BINS))
K1 = 8.0 / BIN_SIZE             # sqrt arg scale
B1 = 1.0 - K1 * D_MIN           # sqrt arg bias
BIG = 1.0e30
C0 = -ALPHA / float(B * HW)     # fold -alpha and global pixel normalizer
CAST_RNE = True

LAST_RESULTS = None


def build_program():
    f32 = mybir.dt.float32
    bf16 = mybir.dt.bfloat16
    i32 = mybir.dt.int32
    Alu = mybir.AluOpType
    Act = mybir.ActivationFunctionType
    Ax = mybir.AxisListType

    nc = bacc_mod.Bacc("TRN2", target_bir_lowering=False)
    # logits kept flat so the gather can index single fp32 elements
    logits = nc.dram_tensor("logits", [C * HW, 1], f32, kind="ExternalInput")
    rowpen = nc.dram_tensor("rowpen", [H, N], f32, kind="ExternalInput")
    colval = nc.dram_tensor("colval", [N, W], f32, kind="ExternalInput")
    ones96 = nc.dram_tensor("ones96", [1, H], f32, kind="ExternalInput")
    pixiota = nc.dram_tensor("pixiota", [H, W], f32, kind="ExternalInput")
    pcolslot = nc.dram_tensor("pcolslot", [128, NGRP], f32, kind="ExternalInput")
    partial = nc.dram_tensor("partial", [128, 1], f32, kind="ExternalOutput")
    tprobe = nc.dram_tensor("tprobe", [1, HW], f32, kind="ExternalOutput")

    lg_full = logits[:, :].rearrange("(c p) o -> c (p o)", c=C)  # (81, HW)

    with ExitStack() as ctx:
        tc = ctx.enter_context(tile.TileContext(nc))
        consts = ctx.enter_context(tc.tile_pool(name="consts", bufs=1))
        rast = ctx.enter_context(tc.tile_pool(name="rast", bufs=1))
        ts_pool = ctx.enter_context(tc.tile_pool(name="tstage", bufs=1))
        lg = ctx.enter_context(tc.tile_pool(name="lg", bufs=3))
        ep = ctx.enter_context(tc.tile_pool(name="ep", bufs=3))
        fin = ctx.enter_context(tc.tile_pool(name="fin", bufs=1))
        psb = ctx.enter_context(tc.tile_pool(name="psb", bufs=2, space="PSUM"))
        pst_pool = ctx.enter_context(tc.tile_pool(name="pst", bufs=4, space="PSUM"))
        dr = ctx.enter_context(tc.tile_pool(name="dr", bufs=1, space="DRAM"))

        # ---- constants
        zero128 = consts.tile([128, 1], f32)
        nc.vector.memset(zero128[:], 0.0)
        nc.const_aps.aps[(f32, 0.0)] = zero128[:]
        b1t = consts.tile([128, 1], f32)
        nc.vector.memset(b1t[:], B1)
        nc.const_aps.aps[(f32, B1)] = b1t[:]

        c_ones96 = consts.tile([1, H], f32)
        nc.sync.dma_start(c_ones96[:], ones96[:, :])
        c_rowpen = consts.tile([H, N], f32)
        nc.sync.dma_start(c_rowpen[:], rowpen[:, :])
        c_pix = consts.tile([H, W], f32)
        nc.sync.dma_start(c_pix[:], pixiota[:, :])
        c_pcs = consts.tile([128, NGRP], f32)
        nc.sync.dma_start(c_pcs[:], pcolslot[:, :])
        c_cvrow = consts.tile([1, N * W], f32)
        nc.sync.dma_start(c_cvrow[:], colval[:, :])
        ident = consts.tile([128, 128], f32)
        masks.make_identity(nc, ident[:])
        identb = consts.tile([128, 128], bf16)
        nc.vector.tensor_copy(out=identb[:], in_=ident[:])

        # ---- rasterize: dmin(h,w) = min_n max(rowpen(h,n), colval(n,w))
        # cv rows broadcast to 96 partitions on gpsimd; two independent
        # min-chains (DVE + gpsimd) halve the serial latency
        dmin = rast.tile([H, W], f32)
        nc.vector.memset(dmin[:], BIG)
        for n in range(N):
            bc = rast.tile([H, W], f32, tag=f"bc{n % 4}")
            nc.gpsimd.partition_broadcast(
                bc[:], c_cvrow[0:1, n * W:(n + 1) * W], channels=H)
            nc.vector.scalar_tensor_tensor(
                out=dmin[:], in0=bc[:], scalar=c_rowpen[:, n:n + 1],
                in1=dmin[:], op0=Alu.max, op1=Alu.min)

        # ---- per-pixel targets in raster layout (96,312)
        fg = ts_pool.tile([H, W], f32)
        nc.vector.tensor_scalar(out=fg[:], in0=dmin[:], scalar1=BIG * 0.5,
                                scalar2=None, op0=Alu.is_lt)
        deff = ts_pool.tile([H, W], f32)
        nc.vector.tensor_tensor(out=deff[:], in0=dmin[:], in1=fg[:], op=Alu.mult)
        sq = ts_pool.tile([H, W], f32)
        nc.scalar.activation(sq[:], deff[:], Act.Sqrt, bias=B1, scale=K1)
        idx = ts_pool.tile([H, W], f32)
        nc.vector.tensor_scalar(out=idx[:], in0=sq[:], scalar1=0.5,
                                scalar2=-0.5, op0=Alu.mult, op1=Alu.add)
        neg = ts_pool.tile([H, W], f32)
        nc.vector.tensor_scalar(out=neg[:], in0=idx[:], scalar1=0.0,
                                scalar2=None, op0=Alu.is_lt)
        idxc = ts_pool.tile([H, W], f32)
        if CAST_RNE:
            nc.vector.tensor_scalar(out=idxc[:], in0=idx[:], scalar1=80.0,
                                    scalar2=-0.5, op0=Alu.min, op1=Alu.add)
        else:
            nc.vector.tensor_scalar(out=idxc[:], in0=idx[:], scalar1=80.0,
                                    scalar2=None, op0=Alu.min)
        ti = ts_pool.tile([H, W], i32)
        nc.vector.tensor_copy(out=ti[:], in_=idxc[:])
        tf = ts_pool.tile([H, W], f32)
        nc.vector.tensor_copy(out=tf[:], in_=ti[:])
        # t = tf + neg*(80 - tf)   (idx<0 -> bin 80)
        d80 = ts_pool.tile([H, W], f32)
        nc.vector.tensor_scalar(out=d80[:], in0=tf[:], scalar1=-1.0,
                                scalar2=80.0, op0=Alu.mult, op1=Alu.add)
        nd = ts_pool.tile([H, W], f32)
        nc.vector.tensor_tensor(out=nd[:], in0=neg[:], in1=d80[:], op=Alu.mult)
        tt_ = ts_pool.tile([H, W], f32)
        nc.vector.tensor_tensor(out=tt_[:], in0=tf[:], in1=nd[:], op=Alu.add)

        nc.scalar.dma_start(tprobe[0:1, :], tt_[:])

        # gather index per pixel: gidx = t*HW + p   (exact in fp32)
        gidxf = ts_pool.tile([H, W], f32)
        nc.vector.scalar_tensor_tensor(
            out=gidxf[:], in0=tt_[:], scalar=float(HW), in1=c_pix[:],
            op0=Alu.mult, op1=Alu.add)
        gidx = ts_pool.tile([H, W], i32)
        nc.vector.tensor_copy(out=gidx[:], in_=gidxf[:])

        # ---- bounce gidx to DRAM (flat pixel order), reload in col-slot
        # layout: slot (i, g) <- pixel 128*g + i
        iscr = dr.tile([NGRP, 128], i32)
        nc.scalar.dma_start(iscr[:, :], gidx[:])
        idx_slot = fin.tile([128, NGRP], i32)
        nc.scalar.dma_start(idx_slot[:], iscr[:, :].rearrange("c i -> i c"))

        # ---- ONE indirect DMA: l_t[i,g] = logits_flat[idx_slot[i,g]]
        lt_slot = fin.tile([128, NGRP], f32)
        nc.gpsimd.indirect_dma_start(
            out=lt_slot[:],
            out_offset=None,
            in_=logits[:, :],
            in_offset=bass.IndirectOffsetOnAxis(ap=idx_slot[:], axis=0),
        )

        # ---- stream logits C-major; exp -> bf16; PE-transpose into PSUM
        # banks; per-bank 3D reduce -> S (128, 234)
        S_dense = fin.tile([128, NGRP], f32)
        bank = None
        for j in range(NCHUNK):
            sl = slice(j * CHUNK, (j + 1) * CHUNK)
            L = lg.tile([C, CHUNK], f32, tag="L")
            if j % 2 == 0:
                nc.sync.dma_start(L[:], lg_full[:, sl])
            else:
                nc.scalar.dma_start(L[:], lg_full[:, sl])
            e = ep.tile([C, CHUNK], f32, tag="e")
            nc.scalar.activation(e[:], L[:], Act.Exp)

            # transpose all groups of chunk j ([18j, 18j+18)) into PSUM
            # banks of 12 groups; reduce each bank when it fills
            for m in range(GP):
                g = j * GP + m
                k = g % GPB
                if k == 0:
                    bank = pst_pool.tile([128, GPB * C], f32, tag="bank")
                nc.tensor.transpose(bank[:, k * C:(k + 1) * C],
                                    e[:, m * 128:(m + 1) * 128],
                                    ident[0:C, 0:C])
                if k == GPB - 1 or g == NGRP - 1:
                    ng = k + 1
                    g0 = g - k
                    nc.vector.tensor_reduce(
                        out=S_dense[:, g0:g0 + ng],
                        in_=bank[:, 0:ng * C].rearrange(
                            "p (g c) -> p g c", g=ng),
                        axis=Ax.X, op=Alu.add)

        # ---- dense epilogue on (128, 234)
        lnS = fin.tile([128, NGRP], f32)
        nc.scalar.activation(lnS[:], S_dense[:], Act.Ln)
        lnp = fin.tile([128, NGRP], f32)
        nc.vector.tensor_tensor(out=lnp[:], in0=lt_slot[:], in1=lnS[:],
                                op=Alu.subtract)
        p = fin.tile([128, NGRP], f32)
        nc.scalar.activation(p[:], lnp[:], Act.Exp)
        om = fin.tile([128, NGRP], f32)
        nc.vector.tensor_scalar(out=om[:], in0=p[:], scalar1=-1.0,
                                scalar2=1.0, op0=Alu.mult, op1=Alu.add)
        om2 = fin.tile([128, NGRP], f32)
        nc.vector.tensor_tensor(out=om2[:], in0=om[:], in1=om[:], op=Alu.mult)
        t3 = fin.tile([128, NGRP], f32)
        nc.vector.tensor_tensor(out=t3[:], in0=om2[:], in1=lnp[:], op=Alu.mult)

        # w from idx: t = (idx - p_colslot)/HW ; w = 1 + 12*(t < 79.5)
        idxf = fin.tile([128, NGRP], f32)
        nc.vector.tensor_copy(out=idxf[:], in_=idx_slot[:])
        tcol = fin.tile([128, NGRP], f32)
        nc.vector.scalar_tensor_tensor(
            out=tcol[:], in0=c_pcs[:], scalar=-1.0, in1=idxf[:],
            op0=Alu.mult, op1=Alu.add)
        wcol = fin.tile([128, NGRP], f32)
        nc.vector.tensor_scalar(out=wcol[:], in0=tcol[:],
                                scalar1=79.5 * float(HW), scalar2=None,
                                op0=Alu.is_lt)
        nc.vector.tensor_scalar(out=wcol[:], in0=wcol[:], scalar1=12.0,
                                scalar2=1.0, op0=Alu.mult, op1=Alu.add)

        acc = fin.tile([128, 1], f32)
        fs = fin.tile([128, NGRP], f32)
        nc.vector.scalar_tensor_tensor(
            out=fs[:], in0=t3[:], scalar=C0, in1=wcol[:],
            op0=Alu.mult, op1=Alu.mult, accum_out=acc[:])
        nc.sync.dma_start(partial[:, :], acc[:])

    nc.compile()
    return nc


_CACHE = {}


def _get_program():
    if "nc" not in _CACHE:
        _CACHE["nc"] = build_program()
    return _CACHE["nc"]


def kernel(depth_logits, gt_boxes2d, num_gt_per_img, gt_center_depth):
    global LAST_RESULTS
    dl = np.ascontiguousarray(np.asarray(depth_logits, dtype=np.float32))
    assert dl.shape == (B, C, H, W), dl.shape
    n_gt = int(num_gt_per_img)
    assert n_gt == N, n_gt
    boxes = np.asarray(gt_boxes2d, dtype=np.float32)
    depth = np.asarray(gt_center_depth, dtype=np.float32)

    u1 = np.floor(boxes[:, 0]).astype(np.int32)
    v1 = np.floor(boxes[:, 1]).astype(np.int32)
    u2 = np.ceil(boxes[:, 2]).astype(np.int32)
    v2 = np.ceil(boxes[:, 3]).astype(np.int32)
    rows = np.arange(H)[:, None]
    cols = np.arange(W)[None, :]
    ones = np.ones((1, H), dtype=np.float32)
    pix = np.arange(HW, dtype=np.float32).reshape(H, W)
    # pixel id of slot (i, g) = 128*g + i
    pcs = (np.arange(NGRP, dtype=np.float32)[None, :] * 128.0
           + np.arange(128, dtype=np.float32)[:, None])

    logits_flat = dl.reshape(B, C * HW, 1)
    in_maps = []
    for b in range(B):
        sl = slice(b * N, (b + 1) * N)
        bv1, bv2, bu1, bu2, d = v1[sl], v2[sl], u1[sl], u2[sl], depth[sl]
        rp = np.where((rows >= bv1[None, :]) & (rows < bv2[None, :]),
                      0.0, BIG).astype(np.float32)              # (H, N)
        cv = np.where((cols >= bu1[:, None]) & (cols < bu2[:, None]),
                      d[:, None], BIG).astype(np.float32)       # (N, W)
        in_maps.append({
            "logits": np.ascontiguousarray(logits_flat[b]),
            "rowpen": np.ascontiguousarray(rp),
            "colval": np.ascontiguousarray(cv),
            "ones96": ones,
            "pixiota": pix,
            "pcolslot": np.ascontiguousarray(pcs),
        })

    nc = _get_program()
    res = run_bass_kernel_spmd(nc, in_maps, core_ids=list(range(B)))
    LAST_RESULTS = res
    total = np.float64(0.0)
    for r in res.results:
        total += np.asarray(r["partial"], dtype=np.float64).sum()
    return np.float32(total)


if __name__ == "__main__":
    import tempfile
    from concourse.bass_utils import compile_bass_kernel
    compile_bass_kernel(_get_program(), tempfile.mkdtemp())
    print("COMPILE OK")
